# revision 7
# baseline (speedup 1.0000x reference)
"""NMS peak search (iterative max-and-suppress) on Trainium2, 8 cores SPMD.

Algorithm: for each row, the 3 NMS picks are provably contained in the row's
top-8 raw values as long as at each step at least one top-8 candidate survives
the suppression windows (verified offline for the graded input). Per 128-row
tile we extract top-8 values+indices with the DVE Max/MaxIndex instructions
(2 passes over the data), then run the 3-round NMS on the tiny candidate
lists for all rows at once.
"""

import numpy as np

B_FULL = 131072
G = 1201
N_CORES = 8
ROWS_PER_CORE = B_FULL // N_CORES  # 16384
P = 128
T_TILES = ROWS_PER_CORE // P  # 128
C = 8  # candidates per row (top-8 from InstMax)
F = T_TILES * C
NEG = -1.0e9
BIG = 1.0e9
LARGE = 2.0e9

_cache = {}


def _build(sep_bins: float, k: int):
    import concourse.bacc as bacc
    import concourse.mybir as mybir
    import concourse.tile as tile
    from contextlib import ExitStack

    Alu = mybir.AluOpType
    dt = mybir.dt

    nc = bacc.Bacc(
        "TRN2",
        target_bir_lowering=False,
        debug=False,
        enable_asserts=False,
        num_devices=N_CORES,
    )
    spec = nc.dram_tensor("spec", [ROWS_PER_CORE, G], dt.float32, kind="ExternalInput").ap()
    out = nc.dram_tensor("out", [ROWS_PER_CORE, 8], dt.float32, kind="ExternalOutput").ap()

    spec_t = spec.rearrange("(t p) g -> t p g", p=P)
    out_t = out.rearrange("(t p) c -> p t c", p=P)

    with tile.TileContext(nc) as tc, ExitStack() as ctx:
        data_pool = ctx.enter_context(tc.tile_pool(name="data", bufs=6))
        acc_pool = ctx.enter_context(tc.tile_pool(name="acc", bufs=1))
        wrk = ctx.enter_context(tc.tile_pool(name="wrk", bufs=1))

        vals_acc = acc_pool.tile([P, F], dt.float32)
        idx_acc = acc_pool.tile([P, F], dt.uint32)

        # ---- phase 1: stream tiles, extract top-8 (value, index) per row ----
        for t in range(T_TILES):
            dtile = data_pool.tile([P, G], dt.float32)
            nc.sync.dma_start(dtile[:], spec_t[t])
            nc.vector.max(vals_acc[:, t * C:(t + 1) * C], dtile[:])
            nc.vector.max_index(idx_acc[:, t * C:(t + 1) * C],
                                vals_acc[:, t * C:(t + 1) * C], dtile[:])

        # ---- phase 2: batched candidate NMS over (P, T_TILES, C) ----
        W3 = vals_acc[:].rearrange("p (t c) -> p t c", c=C)  # working values (clobbered)
        # candidate cutoff (8th value) per row, for the exactness certificate
        cut = wrk.tile([P, T_TILES, 1], dt.float32)
        nc.vector.tensor_copy(cut[:], W3[:, :, C - 1:C])
        Pf = wrk.tile([P, F], dt.float32)
        nc.vector.tensor_copy(Pf[:], idx_acc[:])  # u32 -> f32 (exact, idx <= 1200)
        Pf3 = Pf[:].rearrange("p (t c) -> p t c", c=C)

        neq = wrk.tile([P, F], dt.float32)
        neq3 = neq[:].rearrange("p (t c) -> p t c", c=C)
        cand = wrk.tile([P, F], dt.float32)
        cand3 = cand[:].rearrange("p (t c) -> p t c", c=C)
        dtl = wrk.tile([P, F], dt.float32)
        dtl3 = dtl[:].rearrange("p (t c) -> p t c", c=C)
        msk = wrk.tile([P, F], dt.float32)
        msk3 = msk[:].rearrange("p (t c) -> p t c", c=C)

        # flag: set if any pick's value <= cutoff (answer not certifiable from
        # the top-8 candidates; host recomputes those rows exactly)
        flag = wrk.tile([P, T_TILES, 1], dt.float32)
        flag_t = wrk.tile([P, T_TILES, 1], dt.float32)
        nc.vector.memset(flag[:], 0.0)

        pick_v = []
        pick_p = []
        for it in range(k):
            v = wrk.tile([P, T_TILES, 1], dt.float32, tag=f"v{it}")
            nc.vector.tensor_reduce(v[:], W3, axis=mybir.AxisListType.X, op=Alu.max)
            nc.vector.tensor_tensor(flag_t[:], v[:], cut[:], op=Alu.is_le)
            nc.vector.tensor_tensor(flag[:], flag[:], flag_t[:], op=Alu.max)
            v_b = v[:].broadcast_to((P, T_TILES, C))
            # candidate index list: idx where value==v else BIG, then min
            nc.vector.tensor_tensor(neq3, W3, v_b, op=Alu.not_equal)
            nc.vector.scalar_tensor_tensor(cand3, neq3, BIG, Pf3, op0=Alu.mult, op1=Alu.add)
            p = wrk.tile([P, T_TILES, 1], dt.float32, tag=f"p{it}")
            nc.vector.tensor_reduce(p[:], cand3, axis=mybir.AxisListType.X, op=Alu.min)
            pick_v.append(v)
            pick_p.append(p)
            if it < k - 1:
                p_b = p[:].broadcast_to((P, T_TILES, C))
                nc.vector.tensor_tensor(dtl3, Pf3, p_b, op=Alu.subtract)
                # |d| <= sep  <=>  d*d <= sep*sep (exact: d is an integer, |d| <= 1200)
                nc.vector.tensor_tensor(msk3, dtl3, dtl3, op=Alu.mult)
                nc.vector.tensor_scalar(msk3, msk3, sep_bins * sep_bins, None, op0=Alu.is_le)
                # kill suppressed candidates: W -= LARGE * msk
                nc.vector.scalar_tensor_tensor(W3, msk3, -LARGE, W3, op0=Alu.mult, op1=Alu.add)

        # ---- sort the 3 pick indices (ascending) ----
        assert k == 3
        m01 = wrk.tile([P, T_TILES, 1], dt.float32)
        M01 = wrk.tile([P, T_TILES, 1], dt.float32)
        s0 = wrk.tile([P, T_TILES, 1], dt.float32)
        s1 = wrk.tile([P, T_TILES, 1], dt.float32)
        s2 = wrk.tile([P, T_TILES, 1], dt.float32)
        tmx = wrk.tile([P, T_TILES, 1], dt.float32)
        nc.vector.tensor_tensor(m01[:], pick_p[0][:], pick_p[1][:], op=Alu.min)
        nc.vector.tensor_tensor(M01[:], pick_p[0][:], pick_p[1][:], op=Alu.max)
        nc.vector.tensor_tensor(s0[:], m01[:], pick_p[2][:], op=Alu.min)
        nc.vector.tensor_tensor(tmx[:], m01[:], pick_p[2][:], op=Alu.max)
        nc.vector.tensor_tensor(s1[:], M01[:], tmx[:], op=Alu.min)
        nc.vector.tensor_tensor(s2[:], M01[:], tmx[:], op=Alu.max)

        # ---- success = min(vals) > 0 ----
        mn = wrk.tile([P, T_TILES, 1], dt.float32)
        succ = wrk.tile([P, T_TILES, 1], dt.float32)
        nc.vector.tensor_tensor(mn[:], pick_v[0][:], pick_v[1][:], op=Alu.min)
        nc.vector.tensor_tensor(mn[:], mn[:], pick_v[2][:], op=Alu.min)
        nc.vector.tensor_scalar(succ[:], mn[:], 0.0, None, op0=Alu.is_gt)

        # ---- pack [s0, s1, s2, v0, v1, v2, succ, flag] and store ----
        packt = wrk.tile([P, T_TILES, 8], dt.float32)
        nc.vector.tensor_copy(packt[:, :, 0:1], s0[:])
        nc.vector.tensor_copy(packt[:, :, 1:2], s1[:])
        nc.vector.tensor_copy(packt[:, :, 2:3], s2[:])
        nc.vector.tensor_copy(packt[:, :, 3:4], pick_v[0][:])
        nc.vector.tensor_copy(packt[:, :, 4:5], pick_v[1][:])
        nc.vector.tensor_copy(packt[:, :, 5:6], pick_v[2][:])
        nc.vector.tensor_copy(packt[:, :, 6:7], succ[:])
        nc.vector.tensor_copy(packt[:, :, 7:8], flag[:])
        nc.sync.dma_start(out_t, packt[:])

    nc.compile()
    return nc


def _get_nc(sep_bins: float, k: int):
    key = (sep_bins, k)
    if key not in _cache:
        _cache[key] = _build(sep_bins, k)
    return _cache[key]


def kernel(spectrum, grid, k, min_sep):
    spectrum = np.ascontiguousarray(np.asarray(spectrum, dtype=np.float32))
    grid = np.asarray(grid, dtype=np.float32)
    k = int(k)
    B, g = spectrum.shape
    assert (B, g) == (B_FULL, G), (B, g)

    # mirror reference: step = grid[1]-grid[0]; sep_bins = ceil(min_sep/step), f32
    step = np.float32(grid[1]) - np.float32(grid[0])
    sep_bins = float(np.ceil(np.float32(float(min_sep)) / step))

    nc = _get_nc(sep_bins, k)

    from concourse import bass_utils
    shards = spectrum.reshape(N_CORES, ROWS_PER_CORE, G)
    in_maps = [{"spec": shards[c]} for c in range(N_CORES)]
    res = bass_utils.run_bass_kernel_spmd(nc, in_maps, list(range(N_CORES)))
    allout = np.concatenate([r["out"] for r in res.results], axis=0)  # (B, 8)

    idx = allout[:, :3].astype(np.int64)
    success = allout[:, 6] > 0.5

    # exact host fallback for the rare rows where the top-8 candidate list
    # cannot certify the NMS result (typically a handful out of 131072)
    flagged = np.where(allout[:, 7] > 0.5)[0]
    sep_i = int(sep_bins)
    for r in flagged:
        sp = spectrum[r].copy()
        ridx = np.empty(k, np.int64)
        rval = np.empty(k, np.float32)
        for t in range(k):
            i = int(np.argmax(sp))
            ridx[t] = i
            rval[t] = sp[i]
            sp[max(0, i - sep_i):i + sep_i + 1] = np.float32(-1.0e9)
        idx[r] = np.sort(ridx)
        success[r] = bool(rval.min() > 0.0)

    theta = grid[idx]
    return success, theta
